# revision 1
# baseline (speedup 1.0000x reference)
"""Trainium2 Bass kernel for nn_Bone_loss (VarLoss bone-length variance loss).

Strategy (pure data-parallel over 8 cores, 1024 samples each):
  - The only heavy input is `output` [8192,1,64,64] (134 MB). Each sample
    contributes just 14 gathered scalars (pred at 14 distinct joints), so
    instead of streaming it we use gpsimd dma_gather to fetch one 64-element
    (256 B) chunk per (sample, joint): chunk row = ind>>6; the within-chunk
    offset ind&63 is resolved on-chip with a compare-select against an iota.
  - The critical path is SWDGE descriptor generation on the single Q7 core
    (~8 ns/descriptor, 14336 descriptors in 14 ring-limited calls of 1024).
    Everything else (index math, small-tensor loads, per-chunk selects, bone
    math) is phase-ordered with tile_wait_until to hide under that chain.
  - A dummy 64-idx gather at t~0 prefetches the gpsimd `mlp` ucode library
    (~10 us) under the index-prep path; ap_gather/partition_all_reduce are
    avoided entirely (each would trigger a ~7-20 us library reload) in favor
    of DVE strided copies and a PE ones-matmul.
  - Per-core partial sum -> host adds the 8 partials (the "all-reduce") and
    applies *0.5/B.

Layout (per core, S=1024 samples, halves h in {0,1} of 512):
  sample s = 512*h + 128*b + p   (p = partition, b in [0,4), lane l = 4h+b)
  joint slots j in [0,14) -> joints [0,1,2,3,4,5,6,8,11,12,13,14,15,16]
  gather call (h,k) covers j in {2k,2k+1}: descriptor i = j*512 + s' ->
    G_h[p, 4j+b, 0:64];  int16 row idx = s'*64 + (ind>>6), wrapped
    idx[p16, j*32+u] for s' = 16u+p16 (read from partitions 16-31).
  pred/lo cols: q = h*56 + 4j + b;  bone tensors: cols bone*8 + l.
  Bones are reordered within groups so endpoint pos sequences form affine
  runs (strided-AP copies instead of a gpsimd gather).
"""

import numpy as np

import concourse.bass as bass
import concourse.tile as tile
from concourse import bacc, mybir
from concourse.bass_utils import run_bass_kernel_spmd

NCORES = 8
B = 8192
S = B // NCORES          # samples per core
HS = S // 2              # samples per gather half (int16 row-index limit)
P = 128

_JL = [0, 1, 2, 3, 4, 5, 6, 8, 11, 12, 13, 14, 15, 16]      # joints used
# contiguous joint chunks (jslot0, joint0, cnt); first pair split out for the
# fast-path first gather
_CHUNKS_REST = [(2, 2, 5), (7, 8, 1), (8, 11, 6)]
_CHUNKS_ALL = [(0, 0, 7), (7, 8, 1), (8, 11, 6)]
# Bones reordered within groups so endpoint position sequences form affine
# runs. Groups stay [0:4], [4:8], [8:10], [10:12].
_ID1 = [2, 3, 4, 5, 11, 12, 15, 16, 1, 4, 14, 11]
_ID2 = [1, 2, 5, 6, 12, 13, 14, 15, 0, 0, 8, 8]
_POS = {j: i for i, j in enumerate(_JL)}
_WB = [1.0, 1.0085885098415446, 1.0, 1.0085885098415446,
       1.0, 1.1375361376887123, 1.0, 1.1375361376887123,
       1.0, 1.0, 1.0, 1.0]
# (bone0, len, pos0, stride) affine runs per endpoint; joint0 = _JL[pos0]
_RUNS_E1 = [(0, 4, 2, 1), (4, 2, 8, 1), (6, 2, 12, 1), (8, 1, 1, 1),
            (9, 1, 4, 1), (10, 1, 11, 1), (11, 1, 8, 1)]
_RUNS_E2 = [(0, 2, 1, 1), (2, 2, 5, 1), (4, 4, 9, 1), (8, 2, 0, 0),
            (10, 2, 7, 0)]
_VAR_WEIGHT = 1.0

_F32 = mybir.dt.float32
_I32 = mybir.dt.int32
_I16 = mybir.dt.int16


def _ap(base_ap, dims, off=0):
    """Custom AP: keep base partition dim, override free dims; offset in elems."""
    return bass.AP(base_ap.tensor, base_ap.offset + off,
                   [list(base_ap.ap[0])] + [list(d) for d in dims])


def _dap(base_ap, dims, off=0):
    """Custom DRAM AP with ALL dims explicit (first dim included)."""
    return bass.AP(base_ap.tensor, base_ap.offset + off,
                   [list(d) for d in dims])


def _consts():
    u = np.arange(32, dtype=np.int32)
    p16 = np.arange(16, dtype=np.int32)
    c_base = ((16 * u[None, :] + p16[:, None]) * 64).astype(np.int32)  # [16, 32]
    c_iota = np.broadcast_to(np.arange(64, dtype=np.float32), (P, 64)).copy()
    c_w = np.broadcast_to(np.asarray(_WB, np.float32), (P, 12)).copy()
    c_one = np.ones((P, 1), np.float32)
    return {"c_base": c_base, "c_iota": c_iota, "c_w": c_w,
            "c_one": c_one}


def _build_nc():
    nc = bacc.Bacc("TRN2", target_bir_lowering=False, debug=False,
                   enable_asserts=False, num_devices=NCORES)
    outv = nc.dram_tensor("outv", [S * 64, 64], _F32, kind="ExternalInput").ap()
    indv = nc.dram_tensor("indv", [S, 34], _I32, kind="ExternalInput").ap()
    tgtv = nc.dram_tensor("tgtv", [S, 17], _F32, kind="ExternalInput").ap()
    gxyv = nc.dram_tensor("gxyv", [S, 34], _F32, kind="ExternalInput").ap()
    mskv = nc.dram_tensor("mskv", [S, 17], _F32, kind="ExternalInput").ap()
    c_base = nc.dram_tensor("c_base", [16, 32], _I32, kind="ExternalInput").ap()
    c_iota = nc.dram_tensor("c_iota", [P, 64], _F32, kind="ExternalInput").ap()
    c_w = nc.dram_tensor("c_w", [P, 12], _F32, kind="ExternalInput").ap()
    c_one = nc.dram_tensor("c_one", [P, 1], _F32, kind="ExternalInput").ap()
    res = nc.dram_tensor("res", [1, 1], _F32, kind="ExternalOutput").ap()

    AL = mybir.AluOpType
    X = mybir.AxisListType.X
    with tile.TileContext(nc) as tc:
        with tc.tile_pool(name="sbuf", bufs=1) as pool, \
             tc.tile_pool(name="psum", bufs=1, space="PSUM") as psum_pool:
            # ---------------- phase 0: library prefetch + fast first gather --
            # explicit early load of the gpsimd `mlp` ucode library (the only
            # gpsimd library this kernel uses) so the ~9 us load overlaps the
            # index-prep critical path
            from concourse import library_config
            nc.gpsimd.load_library(library_config.mlp)

            base_t = pool.tile([16, 32], _I32)
            nc.scalar.dma_start(base_t[:], c_base[:])
            iota_t = pool.tile([P, 64], _F32)
            nc.scalar.dma_start(iota_t[:], c_iota[:])
            w_t = pool.tile([P, 12], _F32)
            nc.scalar.dma_start(w_t[:], c_w[:])
            one_t = pool.tile([P, 1], _F32)
            nc.scalar.dma_start(one_t[:], c_one[:])

            idx0 = pool.tile([32, 448], _I16, tag="idx0")
            idx1 = pool.tile([32, 448], _I16, tag="idx1")
            idx_tiles = {0: idx0, 1: idx1}
            g0 = pool.tile([P, 3584], _F32, tag="g0")
            g1 = pool.tile([P, 3584], _F32, tag="g1")
            g_tiles = {0: g0, 1: g1}
            b520 = pool.tile([P, 520], _F32)
            lof = pool.tile([P, 112], _F32)

            # fast path for gather (0,0): joints 0,1 only
            t1c0 = pool.tile([16, 128], _I32)
            nc.sync.dma_start(_ap(t1c0[:], [[4, 32], [1, 4]]),
                              _dap(indv[:], [[34, 16], [544, 32], [1, 4]]))
            ev0 = _ap(t1c0[:], [[4, 32], [2, 2]])
            nc.vector.tensor_scalar(out=ev0, in0=ev0, scalar1=6, scalar2=None,
                                    op0=AL.logical_shift_right)
            nc.vector.tensor_tensor(out=ev0, in0=ev0,
                                    in1=_ap(base_t[:], [[1, 32], [0, 2]]),
                                    op=AL.add)
            nc.vector.tensor_copy(out=_ap(idx0[0:16, :], [[32, 2], [1, 32]]),
                                  in_=_ap(t1c0[:], [[2, 2], [4, 32]]))
            nc.sync.dma_start(idx0[16:32, 0:64], idx0[0:16, 0:64])

            def emit_gather(h, k):
                nc.gpsimd.dma_gather(
                    _ap(g_tiles[h][:], [[64, 8], [1, 64]], off=k * 512),
                    outv[h * HS * 64:(h + 1) * HS * 64, :],
                    idx_tiles[h][0:32, k * 64:(k + 1) * 64],
                    1024, 1024, 64, elem_step=64,
                )

            def emit_stage2(h, k):
                eqt = pool.tile([P, 512], _F32, tag="eq")
                nc.vector.tensor_tensor(
                    out=eqt[:].rearrange("p (a e) -> p a e", e=64),
                    in0=_ap(iota_t[:], [[0, 8], [1, 64]]),
                    in1=_ap(lof[:], [[1, 8], [0, 64]], off=h * 56 + 8 * k),
                    op=AL.is_equal)
                nc.vector.tensor_tensor(
                    out=eqt[:], in0=eqt[:],
                    in1=_ap(g_tiles[h][:], [[1, 512]], off=k * 512), op=AL.mult)
                nc.vector.tensor_reduce(
                    out=_ap(b520[:], [[1, 8]], off=h * 56 + 8 * k),
                    in_=eqt[:].rearrange("p (a e) -> p a e", e=64),
                    axis=X, op=AL.add)

            emit_gather(0, 0)

            # ------------- phase 0.3: batch idx prep (both halves) ----------
            with tc.tile_wait_until(0.3):
                t1raw = pool.tile([16, 2176], _I32)
                for h in range(2):
                    nc.sync.dma_start(
                        _ap(t1raw[:], [[34, 32], [1, 34]], off=h * 1088),
                        _dap(indv[:], [[34, 16], [544, 32], [1, 34]],
                             off=(512 * h) * 34))
                ev = _ap(t1raw[:], [[1088, 2], [34, 32], [2, 17]])
                nc.vector.tensor_scalar(out=ev, in0=ev, scalar1=6, scalar2=None,
                                        op0=AL.logical_shift_right)
                nc.vector.tensor_tensor(
                    out=ev, in0=ev,
                    in1=_ap(base_t[:], [[0, 2], [1, 32], [0, 17]]), op=AL.add)
                for h in range(2):
                    chunks = _CHUNKS_REST if h == 0 else _CHUNKS_ALL
                    it = idx_tiles[h]
                    for (jt, j0, cnt) in chunks:
                        nc.vector.tensor_copy(
                            out=_ap(it[0:16, :], [[32, cnt], [1, 32]], off=jt * 32),
                            in_=_ap(t1raw[:], [[2, cnt], [34, 32]],
                                    off=h * 1088 + 2 * j0))
                    c0 = 64 if h == 0 else 0
                    nc.sync.dma_start(it[16:32, c0:448], it[0:16, c0:448])

            # ------------- phase 0.5: small tensors, lo, active mask --------
            with tc.tile_wait_until(0.5):
                t2raw = pool.tile([P, 272], _I32)
                for h in range(2):
                    nc.scalar.dma_start(
                        _ap(t2raw[:], [[34, 4], [1, 34]], off=h * 136),
                        _dap(indv[:], [[34, 128], [4352, 4], [1, 34]],
                             off=(512 * h) * 34))
                nc.vector.tensor_scalar(out=t2raw[:], in0=t2raw[:], scalar1=63,
                                        scalar2=None, op0=AL.bitwise_and)
                for h in range(2):
                    for (jt, j0, cnt) in _CHUNKS_ALL:
                        nc.vector.tensor_copy(
                            out=_ap(lof[:], [[4, cnt], [1, 4]], off=h * 56 + jt * 4),
                            in_=_ap(t2raw[:], [[2, cnt], [34, 4]],
                                    off=h * 136 + 2 * j0))
                for h in range(2):
                    nc.scalar.dma_start(
                        _ap(b520[:], [[17, 4], [1, 17]], off=112 + h * 68),
                        _dap(tgtv[:], [[17, 128], [2176, 4], [1, 17]],
                             off=(512 * h) * 17))
                    nc.scalar.dma_start(
                        _ap(b520[:], [[34, 4], [1, 34]], off=248 + h * 136),
                        _dap(gxyv[:], [[34, 128], [4352, 4], [1, 34]],
                             off=(512 * h) * 34))
                msk = pool.tile([P, 136], _F32)
                for h in range(2):
                    nc.scalar.dma_start(
                        _ap(msk[:], [[17, 4], [1, 17]], off=h * 68),
                        _dap(mskv[:], [[17, 128], [2176, 4], [1, 17]],
                             off=(512 * h) * 17))
                msum = pool.tile([P, 8], _F32)
                nc.vector.tensor_reduce(out=msum[:],
                                        in_=_ap(msk[:], [[17, 8], [1, 17]]),
                                        axis=X, op=AL.add)
                nc.vector.tensor_scalar(out=msum[:], in0=msum[:], scalar1=0.0,
                                        scalar2=None, op0=AL.is_equal)

            # ------------- gather chain with trailing per-chunk selects -----
            for i in range(1, 14):
                h, k = divmod(i, 7)
                with tc.tile_wait_until(float(i)):
                    emit_gather(h, k)
                hp, kp = divmod(i - 1, 7)
                with tc.tile_wait_until(float(i) + 0.5):
                    emit_stage2(hp, kp)
            with tc.tile_wait_until(14.5):
                emit_stage2(1, 6)

            # ------------- early bone math (target/gt_2d only) --------------
            bg = pool.tile([P, 768], _F32)
            xy2 = pool.tile([P, 96], _F32)
            vis = pool.tile([P, 96], _F32)
            v1 = pool.tile([P, 96], _F32)
            with tc.tile_wait_until(7.5):
                for e, runs in enumerate((_RUNS_E1, _RUNS_E2)):
                    for (b0, ln, q0, st) in runs:
                        j0 = _JL[q0]
                        nc.vector.tensor_copy(
                            out=_ap(bg[:], [[8, ln], [4, 2], [1, 4]],
                                    off=192 + e * 96 + b0 * 8),
                            in_=_ap(b520[:], [[st, ln], [68, 2], [17, 4]],
                                    off=112 + j0))
                        nc.vector.tensor_copy(
                            out=_ap(bg[:], [[16, ln], [8, 2], [4, 2], [1, 4]],
                                    off=384 + e * 192 + b0 * 16),
                            in_=_ap(b520[:], [[2 * st, ln], [1, 2], [136, 2], [34, 4]],
                                    off=248 + 2 * j0))
                n96 = [[1, 96]]
                v2 = pool.tile([P, 96], _F32)
                nc.vector.tensor_scalar(out=v1[:], in0=_ap(bg[:], n96, off=192),
                                        scalar1=0.5, scalar2=None, op0=AL.is_gt)
                nc.vector.tensor_scalar(out=v2[:], in0=_ap(bg[:], n96, off=288),
                                        scalar1=0.5, scalar2=None, op0=AL.is_gt)
                nc.vector.tensor_tensor(out=vis[:], in0=v1[:], in1=v2[:], op=AL.mult)
                dx = pool.tile([P, 96], _F32)
                dy = pool.tile([P, 96], _F32)
                nc.vector.tensor_tensor(
                    out=dx[:].rearrange("p (a b) -> p a b", a=12),
                    in0=_ap(bg[:], [[16, 12], [1, 8]], off=384),
                    in1=_ap(bg[:], [[16, 12], [1, 8]], off=576), op=AL.subtract)
                nc.vector.tensor_tensor(
                    out=dy[:].rearrange("p (a b) -> p a b", a=12),
                    in0=_ap(bg[:], [[16, 12], [1, 8]], off=392),
                    in1=_ap(bg[:], [[16, 12], [1, 8]], off=584), op=AL.subtract)
                nc.vector.tensor_tensor(out=dx[:], in0=dx[:], in1=dx[:], op=AL.mult)
                nc.vector.tensor_tensor(out=dy[:], in0=dy[:], in1=dy[:], op=AL.mult)
                nc.vector.tensor_tensor(out=xy2[:], in0=dx[:], in1=dy[:], op=AL.add)

            # ------------- late bone math (needs pred) ----------------------
            with tc.tile_wait_until(15.0):
                for e, runs in enumerate((_RUNS_E1, _RUNS_E2)):
                    for (b0, ln, q0, st) in runs:
                        nc.vector.tensor_copy(
                            out=_ap(bg[:], [[8, ln], [4, 2], [1, 4]],
                                    off=e * 96 + b0 * 8),
                            in_=_ap(b520[:], [[4 * st, ln], [56, 2], [1, 4]],
                                    off=q0 * 4))
                n96 = [[1, 96]]
                dp = pool.tile([P, 96], _F32)
                nc.vector.tensor_tensor(out=dp[:], in0=_ap(bg[:], n96, off=0),
                                        in1=_ap(bg[:], n96, off=96), op=AL.subtract)
                nc.vector.tensor_tensor(out=dp[:], in0=dp[:], in1=dp[:], op=AL.mult)
                nc.vector.tensor_tensor(out=dp[:], in0=dp[:], in1=xy2[:], op=AL.add)
                ell = pool.tile([P, 96], _F32)
                nc.scalar.sqrt(out=ell[:], in_=dp[:])
                nc.vector.tensor_tensor(
                    out=ell[:].rearrange("p (a b) -> p a b", a=12),
                    in0=ell[:].rearrange("p (a b) -> p a b", a=12),
                    in1=_ap(w_t[:], [[1, 12], [0, 8]]), op=AL.mult)
                nc.vector.tensor_tensor(out=ell[:], in0=ell[:], in1=vis[:],
                                        op=AL.mult)
                # per-group mean E = sum_l / max(num,1) via reciprocal
                sum_l = pool.tile([P, 32], _F32)
                num = pool.tile([P, 32], _F32)
                for (src_t, dst_t) in ((ell, sum_l), (vis, num)):
                    nc.vector.tensor_reduce(
                        out=_ap(dst_t[:], [[8, 2], [1, 8]]),
                        in_=_ap(src_t[:], [[32, 2], [1, 8], [8, 4]]),
                        axis=X, op=AL.add)
                    nc.vector.tensor_reduce(
                        out=_ap(dst_t[:], [[8, 2], [1, 8]], off=16),
                        in_=_ap(src_t[:], [[16, 2], [1, 8], [8, 2]], off=64),
                        axis=X, op=AL.add)
                nc.vector.tensor_scalar(out=num[:], in0=num[:], scalar1=1.0,
                                        scalar2=None, op0=AL.max)
                rn = pool.tile([P, 32], _F32)
                nc.vector.reciprocal(out=rn[:], in_=num[:])
                e_t = pool.tile([P, 32], _F32)
                nc.vector.tensor_tensor(out=e_t[:], in0=sum_l[:], in1=rn[:],
                                        op=AL.mult)
                eb = pool.tile([P, 96], _F32)
                nb = pool.tile([P, 96], _F32)
                for (src_t, dst_t) in ((e_t, eb), (rn, nb)):
                    nc.vector.tensor_copy(
                        out=_ap(dst_t[:], [[32, 2], [8, 4], [1, 8]]),
                        in_=_ap(src_t[:], [[8, 2], [0, 4], [1, 8]]))
                    nc.vector.tensor_copy(
                        out=_ap(dst_t[:], [[16, 2], [8, 2], [1, 8]], off=64),
                        in_=_ap(src_t[:], [[8, 2], [0, 2], [1, 8]], off=16))
                # contrib = gate * (ell-E)^2 * (1/num); global *0.5 on host
                nc.vector.tensor_tensor(out=eb[:], in0=ell[:], in1=eb[:],
                                        op=AL.subtract)
                nc.vector.tensor_tensor(out=eb[:], in0=eb[:], in1=eb[:], op=AL.mult)
                nc.vector.tensor_tensor(out=eb[:], in0=eb[:], in1=nb[:], op=AL.mult)
                nc.vector.tensor_scalar(out=v1[:], in0=ell[:], scalar1=0.0,
                                        scalar2=None, op0=AL.is_gt)
                nc.vector.tensor_tensor(out=v1[:], in0=v1[:], in1=vis[:], op=AL.mult)
                nc.vector.tensor_tensor(out=eb[:], in0=eb[:], in1=v1[:], op=AL.mult)
                # per-lane sums, active mask, cross-partition total via PE
                pl = pool.tile([P, 8], _F32)
                nc.vector.tensor_reduce(out=pl[:],
                                        in_=_ap(eb[:], [[1, 8], [8, 12]]),
                                        axis=X, op=AL.add)
                nc.vector.tensor_tensor(out=pl[:], in0=pl[:], in1=msum[:],
                                        op=AL.mult)
                ps = psum_pool.tile([1, 8], _F32, space="PSUM")
                nc.tensor.matmul(out=ps[:], lhsT=one_t[:], rhs=pl[:],
                                 start=True, stop=True)
                tot = pool.tile([1, 1], _F32)
                nc.vector.tensor_reduce(out=tot[:], in_=ps[:], axis=X, op=AL.add)
                nc.sync.dma_start(res[:], tot[0:1, :])
    nc.compile()
    return nc


_NC_CACHE = None
LAST_RESULTS = None


def kernel(output, mask, ind, target, gt_2d):
    global _NC_CACHE, LAST_RESULTS
    if _NC_CACHE is None:
        _NC_CACHE = _build_nc()
    nc = _NC_CACHE

    output = np.ascontiguousarray(np.asarray(output), dtype=np.float32)
    mask = np.ascontiguousarray(np.asarray(mask), dtype=np.float32)
    target = np.ascontiguousarray(np.asarray(target), dtype=np.float32)
    gt_2d = np.ascontiguousarray(np.asarray(gt_2d), dtype=np.float32)
    ind = np.ascontiguousarray(np.asarray(ind))
    if ind.dtype != np.int64:
        ind = ind.astype(np.int64)

    consts = _consts()
    in_maps = []
    for c in range(NCORES):
        sl = slice(c * S, (c + 1) * S)
        in_maps.append({
            "outv": np.ascontiguousarray(output[sl]).reshape(S * 64, 64),
            "indv": np.ascontiguousarray(ind[sl]).view(np.int32).reshape(S, 34),
            "tgtv": np.ascontiguousarray(target[sl]),
            "gxyv": np.ascontiguousarray(gt_2d[sl]).reshape(S, 34),
            "mskv": np.ascontiguousarray(mask[sl]),
            **consts,
        })
    res = run_bass_kernel_spmd(nc, in_maps, core_ids=list(range(NCORES)))
    LAST_RESULTS = res
    total = sum(float(res.results[c]["res"][0, 0]) for c in range(NCORES))
    return np.asarray([_VAR_WEIGHT * total * 0.5 / B], dtype=np.float32)



# revision 6
# speedup vs baseline: 1.8357x; 1.8357x over previous
"""Trainium2 Bass kernel for nn_Bone_loss (VarLoss bone-length variance loss).

Strategy (pure data-parallel over 8 cores, 1024 samples each):
  - The only heavy input is `output` [8192,1,64,64] (134 MB). Each sample
    contributes just 14 gathered scalars (pred at 14 distinct joints), so
    instead of streaming it we use gpsimd dma_gather to fetch one 64-element
    (256 B) chunk per (sample, joint): chunk row = ind>>6; the within-chunk
    offset ind&63 is resolved on-chip with a compare-select against an iota.
  - The critical path is SWDGE descriptor generation on the single Q7 core
    (~8 ns/descriptor, 14336 descriptors in 14 ring-limited calls of 1024).
    Everything else (index math, small-tensor loads, per-chunk selects, bone
    math) is phase-ordered with tile_wait_until to hide under that chain.
  - A dummy 64-idx gather at t~0 prefetches the gpsimd `mlp` ucode library
    (~10 us) under the index-prep path; ap_gather/partition_all_reduce are
    avoided entirely (each would trigger a ~7-20 us library reload) in favor
    of DVE strided copies and a PE ones-matmul.
  - Per-core partial sum -> host adds the 8 partials (the "all-reduce") and
    applies *0.5/B.

Layout (per core, S=1024 samples, halves h in {0,1} of 512):
  sample s = 512*h + 128*b + p   (p = partition, b in [0,4), lane l = 4h+b)
  joint slots j in [0,14) -> joints [0,1,2,3,4,5,6,8,11,12,13,14,15,16]
  gather call (h,k) covers j in {2k,2k+1}: descriptor i = j*512 + s' ->
    G_h[p, 4j+b, 0:64];  int16 row idx = s'*64 + (ind>>6), wrapped
    idx[p16, j*32+u] for s' = 16u+p16 (read from partitions 16-31).
  pred/lo cols: q = h*56 + 4j + b;  bone tensors: cols bone*8 + l.
  Bones are reordered within groups so endpoint pos sequences form affine
  runs (strided-AP copies instead of a gpsimd gather).
"""

import numpy as np

import concourse.bass as bass
import concourse.tile as tile
from concourse import bacc, mybir
from concourse.bass_utils import run_bass_kernel_spmd

NCORES = 8
B = 8192
S = B // NCORES          # samples per core
HS = S // 2              # samples per gather half (int16 row-index limit)
P = 128

_JL = [0, 1, 2, 3, 4, 5, 6, 8, 11, 12, 13, 14, 15, 16]      # joints used
# contiguous joint chunks (jslot0, joint0, cnt); first pair split out for the
# fast-path first gather
_CHUNKS_REST = [(2, 2, 5), (7, 8, 1), (8, 11, 6)]
_CHUNKS_ALL = [(0, 0, 7), (7, 8, 1), (8, 11, 6)]
# Bones reordered within groups so endpoint position sequences form affine
# runs. Groups stay [0:4], [4:8], [8:10], [10:12].
_ID1 = [2, 3, 4, 5, 11, 12, 15, 16, 1, 4, 14, 11]
_ID2 = [1, 2, 5, 6, 12, 13, 14, 15, 0, 0, 8, 8]
_POS = {j: i for i, j in enumerate(_JL)}
_WB = [1.0, 1.0085885098415446, 1.0, 1.0085885098415446,
       1.0, 1.1375361376887123, 1.0, 1.1375361376887123,
       1.0, 1.0, 1.0, 1.0]
# (bone0, len, pos0, stride) affine runs per endpoint; joint0 = _JL[pos0]
_RUNS_E1 = [(0, 4, 2, 1), (4, 2, 8, 1), (6, 2, 12, 1), (8, 1, 1, 1),
            (9, 1, 4, 1), (10, 1, 11, 1), (11, 1, 8, 1)]
_RUNS_E2 = [(0, 2, 1, 1), (2, 2, 5, 1), (4, 4, 9, 1), (8, 2, 0, 0),
            (10, 2, 7, 0)]
_VAR_WEIGHT = 1.0

_F32 = mybir.dt.float32
_I32 = mybir.dt.int32
_I16 = mybir.dt.int16


def _ap(base_ap, dims, off=0):
    """Custom AP: keep base partition dim, override free dims; offset in elems."""
    return bass.AP(base_ap.tensor, base_ap.offset + off,
                   [list(base_ap.ap[0])] + [list(d) for d in dims])


def _dap(base_ap, dims, off=0):
    """Custom DRAM AP with ALL dims explicit (first dim included)."""
    return bass.AP(base_ap.tensor, base_ap.offset + off,
                   [list(d) for d in dims])


def _consts():
    u = np.arange(32, dtype=np.int32)
    p16 = np.arange(16, dtype=np.int32)
    c_base = ((16 * u[None, :] + p16[:, None]) * 64).astype(np.int32)  # [16, 32]
    c_iota = np.broadcast_to(np.arange(64, dtype=np.float32), (P, 64)).copy()
    c_w = np.broadcast_to(np.asarray(_WB, np.float32), (P, 12)).copy()
    c_one = np.ones((P, 1), np.float32)
    return {"c_base": c_base, "c_iota": c_iota, "c_w": c_w,
            "c_one": c_one}


def _build_nc():
    nc = bacc.Bacc("TRN2", target_bir_lowering=False, debug=False,
                   enable_asserts=False, num_devices=NCORES,
                   num_swdge_queues=4)
    outv = nc.dram_tensor("outv", [S * 64, 64], _F32, kind="ExternalInput").ap()
    indv = nc.dram_tensor("indv", [S, 34], _I32, kind="ExternalInput").ap()
    tgtv = nc.dram_tensor("tgtv", [S, 17], _F32, kind="ExternalInput").ap()
    gxyv = nc.dram_tensor("gxyv", [S, 34], _F32, kind="ExternalInput").ap()
    mskv = nc.dram_tensor("mskv", [S, 17], _F32, kind="ExternalInput").ap()
    c_base = nc.dram_tensor("c_base", [16, 32], _I32, kind="ExternalInput").ap()
    c_iota = nc.dram_tensor("c_iota", [P, 64], _F32, kind="ExternalInput").ap()
    c_w = nc.dram_tensor("c_w", [P, 12], _F32, kind="ExternalInput").ap()
    c_one = nc.dram_tensor("c_one", [P, 1], _F32, kind="ExternalInput").ap()
    res = nc.dram_tensor("res", [1, 1], _F32, kind="ExternalOutput").ap()

    AL = mybir.AluOpType
    X = mybir.AxisListType.X
    with tile.TileContext(nc) as tc:
        with tc.tile_pool(name="sbuf", bufs=1) as pool, \
             tc.tile_pool(name="psum", bufs=1, space="PSUM") as psum_pool:
            # ---------------- phase 0: library prefetch + fast first gather --
            # explicit early load of the gpsimd `mlp` ucode library (the only
            # gpsimd library this kernel uses) so the ~9 us load overlaps the
            # index-prep critical path
            from concourse import library_config
            nc.gpsimd.load_library(library_config.mlp)

            base_t = pool.tile([16, 32], _I32)
            nc.scalar.dma_start(base_t[:], c_base[:])
            iota_t = pool.tile([P, 64], _F32)
            nc.scalar.dma_start(iota_t[:], c_iota[:])
            w_t = pool.tile([P, 12], _F32)
            nc.scalar.dma_start(w_t[:], c_w[:])
            one_t = pool.tile([P, 1], _F32)
            nc.scalar.dma_start(one_t[:], c_one[:])

            idx0 = pool.tile([128, 448], _I16, tag="idx0")
            idx1 = pool.tile([128, 448], _I16, tag="idx1")
            idx_tiles = {0: idx0, 1: idx1}
            g0 = pool.tile([P, 3584], _F32, tag="g0")
            g1 = pool.tile([P, 3584], _F32, tag="g1")
            g_tiles = {0: g0, 1: g1}
            b520 = pool.tile([P, 520], _F32)
            lof = pool.tile([P, 112], _F32)

            # fast path for gather (0,0): joints 0,1 only
            t1c0 = pool.tile([16, 128], _I32)
            nc.sync.dma_start(_ap(t1c0[:], [[4, 32], [1, 4]]),
                              _dap(indv[:], [[34, 16], [544, 32], [1, 4]]))
            ev0 = _ap(t1c0[:], [[4, 32], [2, 2]])
            nc.vector.tensor_scalar(out=ev0, in0=ev0, scalar1=6, scalar2=None,
                                    op0=AL.logical_shift_right)
            nc.vector.tensor_tensor(out=ev0, in0=ev0,
                                    in1=_ap(base_t[:], [[1, 32], [0, 2]]),
                                    op=AL.add)
            nc.vector.tensor_copy(out=_ap(idx0[0:16, :], [[32, 2], [1, 32]]),
                                  in_=_ap(t1c0[:], [[2, 2], [4, 32]]))
            nc.sync.dma_start(idx0[16:32, 0:64], idx0[0:16, 0:64])

            def emit_gather(h, k, q=0):
                # queue q's Q7 core pair reads idxs from partitions
                # [32q, 32q+32); data is replicated across partition groups
                nc.gpsimd.dma_gather(
                    _ap(g_tiles[h][:], [[64, 8], [1, 64]], off=k * 512),
                    outv[h * HS * 64:(h + 1) * HS * 64, :],
                    idx_tiles[h][0:32 * (q + 1), k * 64:(k + 1) * 64],
                    1024, 1024, 64, elem_step=64,
                    queue_num=q,
                )

            def emit_stage2(h, k):
                eqt = pool.tile([P, 512], _F32, tag="eq")
                nc.vector.tensor_tensor(
                    out=eqt[:].rearrange("p (a e) -> p a e", e=64),
                    in0=_ap(iota_t[:], [[0, 8], [1, 64]]),
                    in1=_ap(lof[:], [[1, 8], [0, 64]], off=h * 56 + 8 * k),
                    op=AL.is_equal)
                nc.vector.tensor_tensor(
                    out=eqt[:], in0=eqt[:],
                    in1=_ap(g_tiles[h][:], [[1, 512]], off=k * 512), op=AL.mult)
                nc.vector.tensor_reduce(
                    out=_ap(b520[:], [[1, 8]], off=h * 56 + 8 * k),
                    in_=eqt[:].rearrange("p (a e) -> p a e", e=64),
                    axis=X, op=AL.add)

            emit_gather(0, 0)

            # ------------- phase 0.3: batch idx prep (both halves) ----------
            with tc.tile_wait_until(0.3):
                t1raw = pool.tile([16, 2176], _I32)
                for h in range(2):
                    nc.sync.dma_start(
                        _ap(t1raw[:], [[34, 32], [1, 34]], off=h * 1088),
                        _dap(indv[:], [[34, 16], [544, 32], [1, 34]],
                             off=(512 * h) * 34))
                ev = _ap(t1raw[:], [[1088, 2], [34, 32], [2, 17]])
                nc.vector.tensor_scalar(out=ev, in0=ev, scalar1=6, scalar2=None,
                                        op0=AL.logical_shift_right)
                nc.vector.tensor_tensor(
                    out=ev, in0=ev,
                    in1=_ap(base_t[:], [[0, 2], [1, 32], [0, 17]]), op=AL.add)
                for h in range(2):
                    chunks = _CHUNKS_REST if h == 0 else _CHUNKS_ALL
                    it = idx_tiles[h]
                    for (jt, j0, cnt) in chunks:
                        nc.vector.tensor_copy(
                            out=_ap(it[0:16, :], [[32, cnt], [1, 32]], off=jt * 32),
                            in_=_ap(t1raw[:], [[2, cnt], [34, 32]],
                                    off=h * 1088 + 2 * j0))
                    c0 = 64 if h == 0 else 0
                    nc.sync.dma_start(it[16:32, c0:448], it[0:16, c0:448])
                # replicate idxs to all 128 partitions (queues 1-3 read
                # partition groups [32q, 32q+32))
                for h in range(2):
                    it = idx_tiles[h]
                    nc.sync.dma_start(it[32:64, 0:448], it[0:32, 0:448])
                    nc.sync.dma_start(it[64:128, 0:448], it[0:64, 0:448])

            # ------------- phase 0.5: small tensors, lo, active mask --------
            with tc.tile_wait_until(0.5):
                t2raw = pool.tile([P, 272], _I32)
                for h in range(2):
                    nc.scalar.dma_start(
                        _ap(t2raw[:], [[34, 4], [1, 34]], off=h * 136),
                        _dap(indv[:], [[34, 128], [4352, 4], [1, 34]],
                             off=(512 * h) * 34))
                nc.vector.tensor_scalar(out=t2raw[:], in0=t2raw[:], scalar1=63,
                                        scalar2=None, op0=AL.bitwise_and)
                for h in range(2):
                    for (jt, j0, cnt) in _CHUNKS_ALL:
                        nc.vector.tensor_copy(
                            out=_ap(lof[:], [[4, cnt], [1, 4]], off=h * 56 + jt * 4),
                            in_=_ap(t2raw[:], [[2, cnt], [34, 4]],
                                    off=h * 136 + 2 * j0))
                for h in range(2):
                    nc.scalar.dma_start(
                        _ap(b520[:], [[17, 4], [1, 17]], off=112 + h * 68),
                        _dap(tgtv[:], [[17, 128], [2176, 4], [1, 17]],
                             off=(512 * h) * 17))
                    nc.scalar.dma_start(
                        _ap(b520[:], [[34, 4], [1, 34]], off=248 + h * 136),
                        _dap(gxyv[:], [[34, 128], [4352, 4], [1, 34]],
                             off=(512 * h) * 34))
                msk = pool.tile([P, 136], _F32)
                for h in range(2):
                    nc.scalar.dma_start(
                        _ap(msk[:], [[17, 4], [1, 17]], off=h * 68),
                        _dap(mskv[:], [[17, 128], [2176, 4], [1, 17]],
                             off=(512 * h) * 17))
                msum = pool.tile([P, 8], _F32)
                nc.vector.tensor_reduce(out=msum[:],
                                        in_=_ap(msk[:], [[17, 8], [1, 17]]),
                                        axis=X, op=AL.add)
                nc.vector.tensor_scalar(out=msum[:], in0=msum[:], scalar1=0.0,
                                        scalar2=None, op0=AL.is_equal)

            # ------------- gather chain with trailing per-chunk selects -----
            # round-robin across 4 SWDGE queues (one Q7 core pair each)
            for i in range(1, 14):
                h, k = divmod(i, 7)
                with tc.tile_wait_until(float(i)):
                    emit_gather(h, k, q=i % 4)
                hp, kp = divmod(i - 1, 7)
                with tc.tile_wait_until(float(i) + 0.5):
                    emit_stage2(hp, kp)
            with tc.tile_wait_until(14.5):
                emit_stage2(1, 6)

            # ------------- early bone math (target/gt_2d only) --------------
            bg = pool.tile([P, 768], _F32)
            xy2 = pool.tile([P, 96], _F32)
            vis = pool.tile([P, 96], _F32)
            v1 = pool.tile([P, 96], _F32)
            with tc.tile_wait_until(7.5):
                for e, runs in enumerate((_RUNS_E1, _RUNS_E2)):
                    for (b0, ln, q0, st) in runs:
                        j0 = _JL[q0]
                        nc.vector.tensor_copy(
                            out=_ap(bg[:], [[8, ln], [4, 2], [1, 4]],
                                    off=192 + e * 96 + b0 * 8),
                            in_=_ap(b520[:], [[st, ln], [68, 2], [17, 4]],
                                    off=112 + j0))
                        nc.vector.tensor_copy(
                            out=_ap(bg[:], [[16, ln], [8, 2], [4, 2], [1, 4]],
                                    off=384 + e * 192 + b0 * 16),
                            in_=_ap(b520[:], [[2 * st, ln], [1, 2], [136, 2], [34, 4]],
                                    off=248 + 2 * j0))
                n96 = [[1, 96]]
                v2 = pool.tile([P, 96], _F32)
                nc.vector.tensor_scalar(out=v1[:], in0=_ap(bg[:], n96, off=192),
                                        scalar1=0.5, scalar2=None, op0=AL.is_gt)
                nc.vector.tensor_scalar(out=v2[:], in0=_ap(bg[:], n96, off=288),
                                        scalar1=0.5, scalar2=None, op0=AL.is_gt)
                nc.vector.tensor_tensor(out=vis[:], in0=v1[:], in1=v2[:], op=AL.mult)
                dx = pool.tile([P, 96], _F32)
                dy = pool.tile([P, 96], _F32)
                nc.vector.tensor_tensor(
                    out=dx[:].rearrange("p (a b) -> p a b", a=12),
                    in0=_ap(bg[:], [[16, 12], [1, 8]], off=384),
                    in1=_ap(bg[:], [[16, 12], [1, 8]], off=576), op=AL.subtract)
                nc.vector.tensor_tensor(
                    out=dy[:].rearrange("p (a b) -> p a b", a=12),
                    in0=_ap(bg[:], [[16, 12], [1, 8]], off=392),
                    in1=_ap(bg[:], [[16, 12], [1, 8]], off=584), op=AL.subtract)
                nc.vector.tensor_tensor(out=dx[:], in0=dx[:], in1=dx[:], op=AL.mult)
                nc.vector.tensor_tensor(out=dy[:], in0=dy[:], in1=dy[:], op=AL.mult)
                nc.vector.tensor_tensor(out=xy2[:], in0=dx[:], in1=dy[:], op=AL.add)

            # ------------- late bone math (needs pred) ----------------------
            with tc.tile_wait_until(15.0):
                for e, runs in enumerate((_RUNS_E1, _RUNS_E2)):
                    for (b0, ln, q0, st) in runs:
                        nc.vector.tensor_copy(
                            out=_ap(bg[:], [[8, ln], [4, 2], [1, 4]],
                                    off=e * 96 + b0 * 8),
                            in_=_ap(b520[:], [[4 * st, ln], [56, 2], [1, 4]],
                                    off=q0 * 4))
                n96 = [[1, 96]]
                dp = pool.tile([P, 96], _F32)
                nc.vector.tensor_tensor(out=dp[:], in0=_ap(bg[:], n96, off=0),
                                        in1=_ap(bg[:], n96, off=96), op=AL.subtract)
                nc.vector.tensor_tensor(out=dp[:], in0=dp[:], in1=dp[:], op=AL.mult)
                nc.vector.tensor_tensor(out=dp[:], in0=dp[:], in1=xy2[:], op=AL.add)
                ell = pool.tile([P, 96], _F32)
                nc.scalar.sqrt(out=ell[:], in_=dp[:])
                nc.vector.tensor_tensor(
                    out=ell[:].rearrange("p (a b) -> p a b", a=12),
                    in0=ell[:].rearrange("p (a b) -> p a b", a=12),
                    in1=_ap(w_t[:], [[1, 12], [0, 8]]), op=AL.mult)
                nc.vector.tensor_tensor(out=ell[:], in0=ell[:], in1=vis[:],
                                        op=AL.mult)
                # per-group mean E = sum_l / max(num,1) via reciprocal
                sum_l = pool.tile([P, 32], _F32)
                num = pool.tile([P, 32], _F32)
                for (src_t, dst_t) in ((ell, sum_l), (vis, num)):
                    nc.vector.tensor_reduce(
                        out=_ap(dst_t[:], [[8, 2], [1, 8]]),
                        in_=_ap(src_t[:], [[32, 2], [1, 8], [8, 4]]),
                        axis=X, op=AL.add)
                    nc.vector.tensor_reduce(
                        out=_ap(dst_t[:], [[8, 2], [1, 8]], off=16),
                        in_=_ap(src_t[:], [[16, 2], [1, 8], [8, 2]], off=64),
                        axis=X, op=AL.add)
                nc.vector.tensor_scalar(out=num[:], in0=num[:], scalar1=1.0,
                                        scalar2=None, op0=AL.max)
                rn = pool.tile([P, 32], _F32)
                nc.vector.reciprocal(out=rn[:], in_=num[:])
                e_t = pool.tile([P, 32], _F32)
                nc.vector.tensor_tensor(out=e_t[:], in0=sum_l[:], in1=rn[:],
                                        op=AL.mult)
                eb = pool.tile([P, 96], _F32)
                nb = pool.tile([P, 96], _F32)
                for (src_t, dst_t) in ((e_t, eb), (rn, nb)):
                    nc.vector.tensor_copy(
                        out=_ap(dst_t[:], [[32, 2], [8, 4], [1, 8]]),
                        in_=_ap(src_t[:], [[8, 2], [0, 4], [1, 8]]))
                    nc.vector.tensor_copy(
                        out=_ap(dst_t[:], [[16, 2], [8, 2], [1, 8]], off=64),
                        in_=_ap(src_t[:], [[8, 2], [0, 2], [1, 8]], off=16))
                # contrib = gate * (ell-E)^2 * (1/num); global *0.5 on host
                nc.vector.tensor_tensor(out=eb[:], in0=ell[:], in1=eb[:],
                                        op=AL.subtract)
                nc.vector.tensor_tensor(out=eb[:], in0=eb[:], in1=eb[:], op=AL.mult)
                nc.vector.tensor_tensor(out=eb[:], in0=eb[:], in1=nb[:], op=AL.mult)
                nc.vector.tensor_scalar(out=v1[:], in0=ell[:], scalar1=0.0,
                                        scalar2=None, op0=AL.is_gt)
                nc.vector.tensor_tensor(out=v1[:], in0=v1[:], in1=vis[:], op=AL.mult)
                nc.vector.tensor_tensor(out=eb[:], in0=eb[:], in1=v1[:], op=AL.mult)
                # per-lane sums, active mask, cross-partition total via PE
                pl = pool.tile([P, 8], _F32)
                nc.vector.tensor_reduce(out=pl[:],
                                        in_=_ap(eb[:], [[1, 8], [8, 12]]),
                                        axis=X, op=AL.add)
                nc.vector.tensor_tensor(out=pl[:], in0=pl[:], in1=msum[:],
                                        op=AL.mult)
                ps = psum_pool.tile([1, 8], _F32, space="PSUM")
                nc.tensor.matmul(out=ps[:], lhsT=one_t[:], rhs=pl[:],
                                 start=True, stop=True)
                tot = pool.tile([1, 1], _F32)
                nc.vector.tensor_reduce(out=tot[:], in_=ps[:], axis=X, op=AL.add)
                nc.sync.dma_start(res[:], tot[0:1, :])
    nc.compile()
    return nc


_NC_CACHE = None
LAST_RESULTS = None


def kernel(output, mask, ind, target, gt_2d):
    global _NC_CACHE, LAST_RESULTS
    if _NC_CACHE is None:
        _NC_CACHE = _build_nc()
    nc = _NC_CACHE

    output = np.ascontiguousarray(np.asarray(output), dtype=np.float32)
    mask = np.ascontiguousarray(np.asarray(mask), dtype=np.float32)
    target = np.ascontiguousarray(np.asarray(target), dtype=np.float32)
    gt_2d = np.ascontiguousarray(np.asarray(gt_2d), dtype=np.float32)
    ind = np.ascontiguousarray(np.asarray(ind))
    if ind.dtype != np.int64:
        ind = ind.astype(np.int64)

    consts = _consts()
    in_maps = []
    for c in range(NCORES):
        sl = slice(c * S, (c + 1) * S)
        in_maps.append({
            "outv": np.ascontiguousarray(output[sl]).reshape(S * 64, 64),
            "indv": np.ascontiguousarray(ind[sl]).view(np.int32).reshape(S, 34),
            "tgtv": np.ascontiguousarray(target[sl]),
            "gxyv": np.ascontiguousarray(gt_2d[sl]).reshape(S, 34),
            "mskv": np.ascontiguousarray(mask[sl]),
            **consts,
        })
    res = run_bass_kernel_spmd(nc, in_maps, core_ids=list(range(NCORES)))
    LAST_RESULTS = res
    total = sum(float(res.results[c]["res"][0, 0]) for c in range(NCORES))
    return np.asarray([_VAR_WEIGHT * total * 0.5 / B], dtype=np.float32)



# revision 14
# speedup vs baseline: 2.2240x; 1.2116x over previous
"""Trainium2 Bass kernel for nn_Bone_loss (VarLoss bone-length variance loss).

Strategy (pure data-parallel over 8 cores, 1024 samples each):
  - The only heavy input is `output` [8192,1,64,64] (134 MB). Each sample
    contributes 14 gathered scalars (pred at 14 distinct joints); we use
    gpsimd dma_gather to fetch one 64-element (256 B) chunk per
    (sample, joint): chunk row = ind>>6; the within-chunk offset ind&63 is
    resolved on-chip with an iota-compare mask + multiply + reduce.
  - SWDGE descriptor generation is the critical path (~7-9 ns/descriptor on
    the Q7 pair). We spread it across all 4 SWDGE queues (one Q7 core pair
    each; desc-gen on different queues overlaps on real HW): per half
    (512 samples, int16 row-index limit) the 14 joint slots are split
    across two queues, two calls each (4+3 slots = 2048+1536 descriptors).
  - The within-chunk select is split: the iota-compare masks (independent
    of gathered data) are precomputed into eqm tiles while the gathers run;
    per gather call only a mult + reduce remain.
  - Separate tiles per logical stage (idx, g, eqm, b_pred/b_tgt/b_gxy,
    bg_tv/bg_pred) avoid tile-granularity WAR false dependencies that
    otherwise serialize the tile-framework schedule.
  - Per-core partial sum -> host adds the 8 partials (the "all-reduce") and
    applies *0.5/B.

Layout (per core, S=1024 samples, halves h in {0,1} of 512):
  sample s = 512*h + 128*b + p   (p = partition, b in [0,4), lane l = 4h+b)
  joint slots j in [0,14) -> joints [0,1,2,3,4,5,6,8,11,12,13,14,15,16]
  g_h[p, j*256 + b*64 + e]; idx_h[p16, j*32 + u] for s' = 16u + p16,
  value s'*64 + (ind>>6), replicated over all eight 16-partition groups
  (queue q's core pair reads partitions [32q, 32q+32)).
  pred/lo cols: q = h*56 + 4j + b;  bone tensors: cols bone*8 + l.
  Bones are reordered within groups so endpoint pos sequences form affine
  runs (strided-AP copies instead of a gpsimd gather).
"""

import numpy as np

import concourse.bass as bass
import concourse.tile as tile
from concourse import bacc, mybir
from concourse.bass_utils import run_bass_kernel_spmd

NCORES = 8
B = 8192
S = B // NCORES          # samples per core
HS = S // 2              # samples per gather half (int16 row-index limit)
P = 128

_JL = [0, 1, 2, 3, 4, 5, 6, 8, 11, 12, 13, 14, 15, 16]      # joints used
# contiguous joint chunks (slot0, joint0, cnt) for strided copies
_CHUNKS_ALL = [(0, 0, 7), (7, 8, 1), (8, 11, 6)]
# Bones reordered within groups so endpoint position sequences form affine
# runs. Groups stay [0:4], [4:8], [8:10], [10:12].
_ID1 = [2, 3, 4, 5, 11, 12, 15, 16, 1, 4, 14, 11]
_ID2 = [1, 2, 5, 6, 12, 13, 14, 15, 0, 0, 8, 8]
_POS = {j: i for i, j in enumerate(_JL)}
_WB = [1.0, 1.0085885098415446, 1.0, 1.0085885098415446,
       1.0, 1.1375361376887123, 1.0, 1.1375361376887123,
       1.0, 1.0, 1.0, 1.0]
# (bone0, len, pos0, stride) affine runs per endpoint; joint0 = _JL[pos0]
_RUNS_E1 = [(0, 4, 2, 1), (4, 2, 8, 1), (6, 2, 12, 1), (8, 1, 1, 1),
            (9, 1, 4, 1), (10, 1, 11, 1), (11, 1, 8, 1)]
_RUNS_E2 = [(0, 2, 1, 1), (2, 2, 5, 1), (4, 4, 9, 1), (8, 2, 0, 0),
            (10, 2, 7, 0)]
_VAR_WEIGHT = 1.0

# gather call plan: (half, queue, slot0, nslots); queue q owns 7 slots of
# one half. Calls are capped at 2 slots (1024 descs) by the SWDGE
# descriptor-ring carveout (await_space reserves the whole call up front).
_CALL_ROUNDS = [
    [(0, 0, 0, 2), (0, 1, 7, 2), (1, 2, 0, 2), (1, 3, 7, 2)],
    [(0, 0, 2, 2), (0, 1, 9, 2), (1, 2, 2, 2), (1, 3, 9, 2)],
    [(0, 0, 4, 2), (0, 1, 11, 2), (1, 2, 4, 2), (1, 3, 11, 2)],
    [(0, 0, 6, 1), (0, 1, 13, 1), (1, 2, 6, 1), (1, 3, 13, 1)],
]

_F32 = mybir.dt.float32
_I32 = mybir.dt.int32
_I16 = mybir.dt.int16


def _ap(base_ap, dims, off=0):
    """Custom AP: keep base partition dim, override free dims; offset in elems."""
    return bass.AP(base_ap.tensor, base_ap.offset + off,
                   [list(base_ap.ap[0])] + [list(d) for d in dims])


def _dap(base_ap, dims, off=0):
    """Custom DRAM AP with ALL dims explicit (first dim included)."""
    return bass.AP(base_ap.tensor, base_ap.offset + off,
                   [list(d) for d in dims])


def _consts():
    u = np.arange(32, dtype=np.int32)
    p16 = np.arange(16, dtype=np.int32)
    c_base = ((16 * u[None, :] + p16[:, None]) * 64).astype(np.int32)  # [16, 32]
    c_iota = np.broadcast_to(np.arange(64, dtype=np.float32), (P, 64)).copy()
    c_w = np.broadcast_to(np.asarray(_WB, np.float32), (P, 12)).copy()
    c_one = np.ones((P, 1), np.float32)
    return {"c_base": c_base, "c_iota": c_iota, "c_w": c_w,
            "c_one": c_one}


def _build_nc():
    nc = bacc.Bacc("TRN2", target_bir_lowering=False, debug=False,
                   enable_asserts=False, num_devices=NCORES,
                   num_swdge_queues=4)
    outv = nc.dram_tensor("outv", [S * 64, 64], _F32, kind="ExternalInput").ap()
    indv = nc.dram_tensor("indv", [S, 34], _I32, kind="ExternalInput").ap()
    tgtv = nc.dram_tensor("tgtv", [S, 17], _F32, kind="ExternalInput").ap()
    gxyv = nc.dram_tensor("gxyv", [S, 34], _F32, kind="ExternalInput").ap()
    mskv = nc.dram_tensor("mskv", [S, 17], _F32, kind="ExternalInput").ap()
    c_base = nc.dram_tensor("c_base", [16, 32], _I32, kind="ExternalInput").ap()
    c_iota = nc.dram_tensor("c_iota", [P, 64], _F32, kind="ExternalInput").ap()
    c_w = nc.dram_tensor("c_w", [P, 12], _F32, kind="ExternalInput").ap()
    c_one = nc.dram_tensor("c_one", [P, 1], _F32, kind="ExternalInput").ap()
    res = nc.dram_tensor("res", [1, 1], _F32, kind="ExternalOutput").ap()

    AL = mybir.AluOpType
    X = mybir.AxisListType.X
    with tile.TileContext(nc) as tc:
        with tc.tile_pool(name="sbuf", bufs=1) as pool, \
             tc.tile_pool(name="psum", bufs=1, space="PSUM") as psum_pool:
            # ---------------- phase 0: library prefetch + const loads --------
            from concourse import library_config
            nc.gpsimd.load_library(library_config.mlp)

            base_t = pool.tile([16, 32], _I32)
            nc.scalar.dma_start(base_t[:], c_base[:])
            iota_t = pool.tile([P, 64], _F32)
            nc.scalar.dma_start(iota_t[:], c_iota[:])
            w_t = pool.tile([P, 12], _F32)
            nc.scalar.dma_start(w_t[:], c_w[:])
            one_t = pool.tile([P, 1], _F32)
            nc.scalar.dma_start(one_t[:], c_one[:])

            idx0 = pool.tile([128, 448], _I16, tag="idx0")
            idx1 = pool.tile([128, 448], _I16, tag="idx1")
            idx_tiles = {0: idx0, 1: idx1}
            g0 = pool.tile([P, 3584], _F32, tag="g0")
            g1 = pool.tile([P, 3584], _F32, tag="g1")
            g_tiles = {0: g0, 1: g1}
            eqm0 = pool.tile([P, 3584], _F32, tag="eqm0")
            eqm1 = pool.tile([P, 3584], _F32, tag="eqm1")
            eqm_tiles = {0: eqm0, 1: eqm1}
            b_pred = pool.tile([P, 112], _F32)
            lof = pool.tile([P, 112], _F32)
            b_tgt = pool.tile([P, 136], _F32)
            b_gxy = pool.tile([P, 272], _F32)

            # ------------- phase 0.2: idx prep (both halves) ----------------
            with tc.tile_wait_until(0.2):
                t1raw = pool.tile([16, 2176], _I32)
                for h in range(2):
                    nc.sync.dma_start(
                        _ap(t1raw[:], [[34, 32], [1, 34]], off=h * 1088),
                        _dap(indv[:], [[34, 16], [544, 32], [1, 34]],
                             off=(512 * h) * 34))
                ev = _ap(t1raw[:], [[1088, 2], [34, 32], [2, 17]])
                nc.vector.tensor_scalar(out=ev, in0=ev, scalar1=6, scalar2=None,
                                        op0=AL.logical_shift_right)
                nc.vector.tensor_tensor(
                    out=ev, in0=ev,
                    in1=_ap(base_t[:], [[0, 2], [1, 32], [0, 17]]), op=AL.add)
                for h in range(2):
                    it = idx_tiles[h]
                    for (jt, j0, cnt) in _CHUNKS_ALL:
                        nc.vector.tensor_copy(
                            out=_ap(it[0:16, :], [[32, cnt], [1, 32]], off=jt * 32),
                            in_=_ap(t1raw[:], [[2, cnt], [34, 32]],
                                    off=h * 1088 + 2 * j0))
                    nc.sync.dma_start(it[16:32, 0:448], it[0:16, 0:448])
                # replicate to all 128 partitions (queue q's pair reads
                # partitions [32q, 32q+32)); three parallel copies of [0:32]
                for h in range(2):
                    it = idx_tiles[h]
                    nc.sync.dma_start(it[32:64, 0:448], it[0:32, 0:448])
                    nc.scalar.dma_start(it[64:96, 0:448], it[0:32, 0:448])
                    nc.sync.dma_start(it[96:128, 0:448], it[0:32, 0:448])

            # ------------- phase 0.3: small tensors, lo, active mask --------
            with tc.tile_wait_until(0.3):
                t2raw = pool.tile([P, 272], _I32)
                for h in range(2):
                    nc.scalar.dma_start(
                        _ap(t2raw[:], [[34, 4], [1, 34]], off=h * 136),
                        _dap(indv[:], [[34, 128], [4352, 4], [1, 34]],
                             off=(512 * h) * 34))
                nc.vector.tensor_scalar(out=t2raw[:], in0=t2raw[:], scalar1=63,
                                        scalar2=None, op0=AL.bitwise_and)
                for h in range(2):
                    for (jt, j0, cnt) in _CHUNKS_ALL:
                        nc.vector.tensor_copy(
                            out=_ap(lof[:], [[4, cnt], [1, 4]], off=h * 56 + jt * 4),
                            in_=_ap(t2raw[:], [[2, cnt], [34, 4]],
                                    off=h * 136 + 2 * j0))
                for h in range(2):
                    nc.scalar.dma_start(
                        _ap(b_tgt[:], [[17, 4], [1, 17]], off=h * 68),
                        _dap(tgtv[:], [[17, 128], [2176, 4], [1, 17]],
                             off=(512 * h) * 17))
                    nc.scalar.dma_start(
                        _ap(b_gxy[:], [[34, 4], [1, 34]], off=h * 136),
                        _dap(gxyv[:], [[34, 128], [4352, 4], [1, 34]],
                             off=(512 * h) * 34))
                msk = pool.tile([P, 136], _F32)
                for h in range(2):
                    nc.scalar.dma_start(
                        _ap(msk[:], [[17, 4], [1, 17]], off=h * 68),
                        _dap(mskv[:], [[17, 128], [2176, 4], [1, 17]],
                             off=(512 * h) * 17))
                msum = pool.tile([P, 8], _F32)
                nc.vector.tensor_reduce(out=msum[:],
                                        in_=_ap(msk[:], [[17, 8], [1, 17]]),
                                        axis=X, op=AL.add)
                nc.vector.tensor_scalar(out=msum[:], in0=msum[:], scalar1=0.0,
                                        scalar2=None, op0=AL.is_equal)

            # ------------- phase 0.5: eq-mask precompute (hidden) -----------
            with tc.tile_wait_until(0.5):
                for h in range(2):
                    nc.vector.tensor_tensor(
                        out=eqm_tiles[h][:].rearrange("p (a e) -> p a e", e=64),
                        in0=_ap(iota_t[:], [[0, 56], [1, 64]]),
                        in1=_ap(lof[:], [[1, 56], [0, 64]], off=h * 56),
                        op=AL.is_equal)

            # ------------- gathers: 2 rounds x 4 queues ---------------------
            def emit_gather(h, q, s0, ns, ph):
                with tc.tile_wait_until(ph):
                    nc.gpsimd.dma_gather(
                        _ap(g_tiles[h][:], [[64, ns * 4], [1, 64]], off=s0 * 256),
                        outv[h * HS * 64:(h + 1) * HS * 64, :],
                        idx_tiles[h][0:32 * (q + 1), s0 * 32:(s0 + ns) * 32],
                        ns * 512, ns * 512, 64, elem_step=64,
                        queue_num=q,
                    )

            def emit_select(h, s0, ns, ph):
                # eqm <- eqm * g (in place), then reduce over e -> b_pred
                with tc.tile_wait_until(ph):
                    eview = _ap(eqm_tiles[h][:], [[1, ns * 256]], off=s0 * 256)
                    nc.vector.tensor_tensor(
                        out=eview, in0=eview,
                        in1=_ap(g_tiles[h][:], [[1, ns * 256]], off=s0 * 256),
                        op=AL.mult)
                    nc.vector.tensor_reduce(
                        out=_ap(b_pred[:], [[1, ns * 4]], off=h * 56 + s0 * 4),
                        in_=_ap(eqm_tiles[h][:], [[64, ns * 4], [1, 64]],
                                off=s0 * 256),
                        axis=X, op=AL.add)

            for ri, calls in enumerate(_CALL_ROUNDS):
                for ci, (h, q, s0, ns) in enumerate(calls):
                    emit_gather(h, q, s0, ns, 1.0 + ri + 0.01 * ci)
                for ci, (h, q, s0, ns) in enumerate(calls):
                    emit_select(h, s0, ns, 1.5 + ri + 0.01 * ci)

            # ------------- early bone math (target/gt_2d only) --------------
            bg_tv = pool.tile([P, 576], _F32)
            bg_pred = pool.tile([P, 192], _F32)
            xy2 = pool.tile([P, 96], _F32)
            vw = pool.tile([P, 96], _F32)
            rn = pool.tile([P, 32], _F32)
            with tc.tile_wait_until(0.6):
                for e, runs in enumerate((_RUNS_E1, _RUNS_E2)):
                    for (b0, ln, q0, st) in runs:
                        j0 = _JL[q0]
                        nc.vector.tensor_copy(
                            out=_ap(bg_tv[:], [[8, ln], [4, 2], [1, 4]],
                                    off=e * 96 + b0 * 8),
                            in_=_ap(b_tgt[:], [[st, ln], [68, 2], [17, 4]],
                                    off=j0))
                        nc.vector.tensor_copy(
                            out=_ap(bg_tv[:], [[16, ln], [8, 2], [4, 2], [1, 4]],
                                    off=192 + e * 192 + b0 * 16),
                            in_=_ap(b_gxy[:], [[2 * st, ln], [1, 2], [136, 2],
                                               [34, 4]],
                                    off=2 * j0))
                n96 = [[1, 96]]
                vis = pool.tile([P, 96], _F32)
                v2 = pool.tile([P, 96], _F32)
                nc.vector.tensor_scalar(out=vis[:], in0=_ap(bg_tv[:], n96, off=0),
                                        scalar1=0.5, scalar2=None, op0=AL.is_gt)
                nc.vector.tensor_scalar(out=v2[:], in0=_ap(bg_tv[:], n96, off=96),
                                        scalar1=0.5, scalar2=None, op0=AL.is_gt)
                nc.vector.tensor_tensor(out=vis[:], in0=vis[:], in1=v2[:],
                                        op=AL.mult)
                # vw = vis * w  (fold bone weight into the visibility gate)
                nc.vector.tensor_tensor(
                    out=vw[:].rearrange("p (a b) -> p a b", a=12),
                    in0=vis[:].rearrange("p (a b) -> p a b", a=12),
                    in1=_ap(w_t[:], [[1, 12], [0, 8]]), op=AL.mult)
                dx = pool.tile([P, 96], _F32)
                dy = pool.tile([P, 96], _F32)
                nc.vector.tensor_tensor(
                    out=dx[:].rearrange("p (a b) -> p a b", a=12),
                    in0=_ap(bg_tv[:], [[16, 12], [1, 8]], off=192),
                    in1=_ap(bg_tv[:], [[16, 12], [1, 8]], off=384), op=AL.subtract)
                nc.vector.tensor_tensor(
                    out=dy[:].rearrange("p (a b) -> p a b", a=12),
                    in0=_ap(bg_tv[:], [[16, 12], [1, 8]], off=200),
                    in1=_ap(bg_tv[:], [[16, 12], [1, 8]], off=392), op=AL.subtract)
                nc.vector.tensor_tensor(out=dx[:], in0=dx[:], in1=dx[:], op=AL.mult)
                nc.vector.tensor_tensor(out=dy[:], in0=dy[:], in1=dy[:], op=AL.mult)
                nc.vector.tensor_tensor(out=xy2[:], in0=dx[:], in1=dy[:], op=AL.add)
                # num = bones visible per group; rn = 1/max(num, 1)
                num = pool.tile([P, 32], _F32)
                nc.vector.tensor_reduce(
                    out=_ap(num[:], [[8, 2], [1, 8]]),
                    in_=_ap(vis[:], [[32, 2], [1, 8], [8, 4]]),
                    axis=X, op=AL.add)
                nc.vector.tensor_reduce(
                    out=_ap(num[:], [[8, 2], [1, 8]], off=16),
                    in_=_ap(vis[:], [[16, 2], [1, 8], [8, 2]], off=64),
                    axis=X, op=AL.add)
                nc.vector.tensor_scalar(out=num[:], in0=num[:], scalar1=1.0,
                                        scalar2=None, op0=AL.max)
                nc.vector.reciprocal(out=rn[:], in_=num[:])

            # ------------- late bone math (needs pred) ----------------------
            with tc.tile_wait_until(6.0):
                for e, runs in enumerate((_RUNS_E1, _RUNS_E2)):
                    for (b0, ln, q0, st) in runs:
                        nc.vector.tensor_copy(
                            out=_ap(bg_pred[:], [[8, ln], [4, 2], [1, 4]],
                                    off=e * 96 + b0 * 8),
                            in_=_ap(b_pred[:], [[4 * st, ln], [56, 2], [1, 4]],
                                    off=q0 * 4))
                n96 = [[1, 96]]
                dp = pool.tile([P, 96], _F32)
                nc.vector.tensor_tensor(out=dp[:], in0=_ap(bg_pred[:], n96, off=0),
                                        in1=_ap(bg_pred[:], n96, off=96),
                                        op=AL.subtract)
                nc.vector.tensor_tensor(out=dp[:], in0=dp[:], in1=dp[:], op=AL.mult)
                nc.vector.tensor_tensor(out=dp[:], in0=dp[:], in1=xy2[:], op=AL.add)
                ell = pool.tile([P, 96], _F32)
                nc.scalar.sqrt(out=ell[:], in_=dp[:])
                nc.vector.tensor_tensor(out=ell[:], in0=ell[:], in1=vw[:],
                                        op=AL.mult)
                # per-group mean E = sum_l * rn
                sum_l = pool.tile([P, 32], _F32)
                nc.vector.tensor_reduce(
                    out=_ap(sum_l[:], [[8, 2], [1, 8]]),
                    in_=_ap(ell[:], [[32, 2], [1, 8], [8, 4]]),
                    axis=X, op=AL.add)
                nc.vector.tensor_reduce(
                    out=_ap(sum_l[:], [[8, 2], [1, 8]], off=16),
                    in_=_ap(ell[:], [[16, 2], [1, 8], [8, 2]], off=64),
                    axis=X, op=AL.add)
                e_t = pool.tile([P, 32], _F32)
                nc.vector.tensor_tensor(out=e_t[:], in0=sum_l[:], in1=rn[:],
                                        op=AL.mult)
                # d = ell - E (broadcast E/rn back to bones via stride-0 APs)
                eb = pool.tile([P, 96], _F32)
                nc.vector.tensor_tensor(
                    out=_ap(eb[:], [[32, 2], [8, 4], [1, 8]]),
                    in0=_ap(ell[:], [[32, 2], [8, 4], [1, 8]]),
                    in1=_ap(e_t[:], [[8, 2], [0, 4], [1, 8]]), op=AL.subtract)
                nc.vector.tensor_tensor(
                    out=_ap(eb[:], [[16, 2], [8, 2], [1, 8]], off=64),
                    in0=_ap(ell[:], [[16, 2], [8, 2], [1, 8]], off=64),
                    in1=_ap(e_t[:], [[8, 2], [0, 2], [1, 8]], off=16),
                    op=AL.subtract)
                nc.vector.tensor_tensor(out=eb[:], in0=eb[:], in1=eb[:], op=AL.mult)
                nc.vector.tensor_tensor(
                    out=_ap(eb[:], [[32, 2], [8, 4], [1, 8]]),
                    in0=_ap(eb[:], [[32, 2], [8, 4], [1, 8]]),
                    in1=_ap(rn[:], [[8, 2], [0, 4], [1, 8]]), op=AL.mult)
                nc.vector.tensor_tensor(
                    out=_ap(eb[:], [[16, 2], [8, 2], [1, 8]], off=64),
                    in0=_ap(eb[:], [[16, 2], [8, 2], [1, 8]], off=64),
                    in1=_ap(rn[:], [[8, 2], [0, 2], [1, 8]], off=16), op=AL.mult)
                # gate = (ell > 0): l>0 implies vis (l==0 when invisible)
                gt = pool.tile([P, 96], _F32)
                nc.vector.tensor_scalar(out=gt[:], in0=ell[:], scalar1=0.0,
                                        scalar2=None, op0=AL.is_gt)
                nc.vector.tensor_tensor(out=eb[:], in0=eb[:], in1=gt[:],
                                        op=AL.mult)
                # per-lane sums, active mask, cross-partition total via PE
                pl = pool.tile([P, 8], _F32)
                nc.vector.tensor_reduce(out=pl[:],
                                        in_=_ap(eb[:], [[1, 8], [8, 12]]),
                                        axis=X, op=AL.add)
                nc.vector.tensor_tensor(out=pl[:], in0=pl[:], in1=msum[:],
                                        op=AL.mult)
                ps = psum_pool.tile([1, 8], _F32, space="PSUM")
                nc.tensor.matmul(out=ps[:], lhsT=one_t[:], rhs=pl[:],
                                 start=True, stop=True)
                tot = pool.tile([1, 1], _F32)
                nc.vector.tensor_reduce(out=tot[:], in_=ps[:], axis=X, op=AL.add)
                nc.sync.dma_start(res[:], tot[0:1, :])
    nc.compile()
    return nc


_NC_CACHE = None
LAST_RESULTS = None


def kernel(output, mask, ind, target, gt_2d):
    global _NC_CACHE, LAST_RESULTS
    if _NC_CACHE is None:
        _NC_CACHE = _build_nc()
    nc = _NC_CACHE

    output = np.ascontiguousarray(np.asarray(output), dtype=np.float32)
    mask = np.ascontiguousarray(np.asarray(mask), dtype=np.float32)
    target = np.ascontiguousarray(np.asarray(target), dtype=np.float32)
    gt_2d = np.ascontiguousarray(np.asarray(gt_2d), dtype=np.float32)
    ind = np.ascontiguousarray(np.asarray(ind))
    if ind.dtype != np.int64:
        ind = ind.astype(np.int64)

    consts = _consts()
    in_maps = []
    for c in range(NCORES):
        sl = slice(c * S, (c + 1) * S)
        in_maps.append({
            "outv": np.ascontiguousarray(output[sl]).reshape(S * 64, 64),
            "indv": np.ascontiguousarray(ind[sl]).view(np.int32).reshape(S, 34),
            "tgtv": np.ascontiguousarray(target[sl]),
            "gxyv": np.ascontiguousarray(gt_2d[sl]).reshape(S, 34),
            "mskv": np.ascontiguousarray(mask[sl]),
            **consts,
        })
    res = run_bass_kernel_spmd(nc, in_maps, core_ids=list(range(NCORES)))
    LAST_RESULTS = res
    total = sum(float(res.results[c]["res"][0, 0]) for c in range(NCORES))
    return np.asarray([_VAR_WEIGHT * total * 0.5 / B], dtype=np.float32)


# revision 16
# speedup vs baseline: 2.4024x; 1.0802x over previous
"""Trainium2 Bass kernel for nn_Bone_loss (VarLoss bone-length variance loss).

Strategy (pure data-parallel over 8 cores, 1024 samples each):
  - The only heavy input is `output` [8192,1,64,64] (134 MB). Each sample
    contributes 14 gathered scalars (pred at 14 distinct joints); we use
    gpsimd dma_gather to fetch one 64-element (256 B) chunk per
    (sample, joint): chunk row = ind>>6; the within-chunk offset ind&63 is
    resolved on-chip with an iota-compare mask + multiply + reduce.
  - SWDGE descriptor generation is the critical path (~7-9 ns/descriptor on
    a Q7 pair). It is spread across all 4 SWDGE queues (one Q7 core pair
    each; desc-gen on different queues overlaps on real HW, retirement is
    in order). Per half (512 samples, int16 row-index limit) the 14 joint
    slots split across two queues; calls are capped at 2 slots (1024
    descriptors) by the SWDGE ring carveout.
  - The iota-compare masks (independent of gathered data) are precomputed
    into eqm tiles while the gathers run; per gather call only a
    mult + reduce remain on DVE.
  - Input loads are split across the sync and scalar HWDGE queues ordered
    by when their consumers run; idx int16 data is replicated to all 128
    partitions (queue q's core pair reads partitions [32q, 32q+32)).
  - Separate tiles per logical stage avoid tile-granularity WAR false
    dependencies.
  - Per-core partial sum -> host adds the 8 partials (the "all-reduce") and
    applies *0.5/B.

Layout (per core, S=1024 samples, halves h in {0,1} of 512):
  sample s = 512*h + 128*b + p   (p = partition, b in [0,4), lane l = 4h+b)
  joint slots j in [0,14) -> joints [0,1,2,3,4,5,6,8,11,12,13,14,15,16]
  g_h[p, j*256 + b*64 + e]; idx_h[p16, j*32 + u] for s' = 16u + p16,
  value s'*64 + (ind>>6), replicated over all eight 16-partition groups.
  pred/lo cols: q = h*56 + 4j + b;  bone tensors: cols bone*8 + l.
  Bones are reordered within groups so endpoint pos sequences form affine
  runs (strided-AP reads instead of a gpsimd gather).
"""

import numpy as np

import concourse.bass as bass
import concourse.tile as tile
from concourse import bacc, mybir
from concourse.bass_utils import run_bass_kernel_spmd

NCORES = 8
B = 8192
S = B // NCORES          # samples per core
HS = S // 2              # samples per gather half (int16 row-index limit)
P = 128

_JL = [0, 1, 2, 3, 4, 5, 6, 8, 11, 12, 13, 14, 15, 16]      # joints used
# contiguous joint chunks (slot0, joint0, cnt) for strided copies
_CHUNKS_ALL = [(0, 0, 7), (7, 8, 1), (8, 11, 6)]
# Bones reordered within groups so endpoint position sequences form affine
# runs. Groups stay [0:4], [4:8], [8:10], [10:12].
_ID1 = [2, 3, 4, 5, 11, 12, 15, 16, 1, 4, 14, 11]
_ID2 = [1, 2, 5, 6, 12, 13, 14, 15, 0, 0, 8, 8]
_POS = {j: i for i, j in enumerate(_JL)}
_WB = [1.0, 1.0085885098415446, 1.0, 1.0085885098415446,
       1.0, 1.1375361376887123, 1.0, 1.1375361376887123,
       1.0, 1.0, 1.0, 1.0]
# (bone0, len, pos0, stride) affine runs per endpoint; joint0 = _JL[pos0]
_RUNS_E1 = [(0, 4, 2, 1), (4, 2, 8, 1), (6, 2, 12, 1), (8, 1, 1, 1),
            (9, 1, 4, 1), (10, 1, 11, 1), (11, 1, 8, 1)]
_RUNS_E2 = [(0, 2, 1, 1), (2, 2, 5, 1), (4, 4, 9, 1), (8, 2, 0, 0),
            (10, 2, 7, 0)]
# merged E1/E2 runs (bone0, len, e1pos0, e1st, e2pos0, e2st) for the late
# pred-difference: dp[bone range] = pred[E1 pos seq] - pred[E2 pos seq]
_RUNS_D = [(0, 2, 2, 1, 1, 1), (2, 2, 4, 1, 5, 1), (4, 2, 8, 1, 9, 1),
           (6, 2, 12, 1, 11, 1), (8, 1, 1, 1, 0, 1), (9, 1, 4, 1, 0, 1),
           (10, 1, 11, 1, 7, 1), (11, 1, 8, 1, 7, 1)]
_VAR_WEIGHT = 1.0

# gather call plan: (half, queue, slot0, nslots); queue q owns 7 slots of
# one half. Calls are capped at 2 slots (1024 descs) by the SWDGE
# descriptor-ring carveout (await_space reserves the whole call up front).
_CALL_ROUNDS = [
    [(0, 0, 0, 2), (0, 1, 7, 2), (1, 2, 0, 2), (1, 3, 7, 2)],
    [(0, 0, 2, 2), (0, 1, 9, 2), (1, 2, 2, 2), (1, 3, 9, 2)],
    [(0, 0, 4, 2), (0, 1, 11, 2), (1, 2, 4, 2), (1, 3, 11, 2)],
    [(0, 0, 6, 1), (0, 1, 13, 1), (1, 2, 6, 1), (1, 3, 13, 1)],
]

_F32 = mybir.dt.float32
_I32 = mybir.dt.int32
_I16 = mybir.dt.int16


def _ap(base_ap, dims, off=0):
    """Custom AP: keep base partition dim, override free dims; offset in elems."""
    return bass.AP(base_ap.tensor, base_ap.offset + off,
                   [list(base_ap.ap[0])] + [list(d) for d in dims])


def _dap(base_ap, dims, off=0):
    """Custom DRAM AP with ALL dims explicit (first dim included)."""
    return bass.AP(base_ap.tensor, base_ap.offset + off,
                   [list(d) for d in dims])


def _consts():
    u = np.arange(32, dtype=np.int32)
    p16 = np.arange(16, dtype=np.int32)
    c_base = ((16 * u[None, :] + p16[:, None]) * 64).astype(np.int32)  # [16, 32]
    c_iota = np.broadcast_to(np.arange(64, dtype=np.float32), (P, 64)).copy()
    c_w = np.broadcast_to(np.asarray(_WB, np.float32), (P, 12)).copy()
    c_one = np.ones((P, 1), np.float32)
    return {"c_base": c_base, "c_iota": c_iota, "c_w": c_w,
            "c_one": c_one}


def _build_nc():
    nc = bacc.Bacc("TRN2", target_bir_lowering=False, debug=False,
                   enable_asserts=False, num_devices=NCORES,
                   num_swdge_queues=4)
    outv = nc.dram_tensor("outv", [S * 64, 64], _F32, kind="ExternalInput").ap()
    indv = nc.dram_tensor("indv", [S, 34], _I32, kind="ExternalInput").ap()
    tgtv = nc.dram_tensor("tgtv", [S, 17], _F32, kind="ExternalInput").ap()
    gxyv = nc.dram_tensor("gxyv", [S, 34], _F32, kind="ExternalInput").ap()
    mskv = nc.dram_tensor("mskv", [S, 17], _F32, kind="ExternalInput").ap()
    c_base = nc.dram_tensor("c_base", [16, 32], _I32, kind="ExternalInput").ap()
    c_iota = nc.dram_tensor("c_iota", [P, 64], _F32, kind="ExternalInput").ap()
    c_w = nc.dram_tensor("c_w", [P, 12], _F32, kind="ExternalInput").ap()
    c_one = nc.dram_tensor("c_one", [P, 1], _F32, kind="ExternalInput").ap()
    res = nc.dram_tensor("res", [1, 1], _F32, kind="ExternalOutput").ap()

    AL = mybir.AluOpType
    X = mybir.AxisListType.X
    with tile.TileContext(nc) as tc:
        with tc.tile_pool(name="sbuf", bufs=1) as pool, \
             tc.tile_pool(name="psum", bufs=1, space="PSUM") as psum_pool:
            from concourse import library_config
            nc.gpsimd.load_library(library_config.mlp)

            base_t = pool.tile([16, 32], _I32)
            iota_t = pool.tile([P, 64], _F32)
            w_t = pool.tile([P, 12], _F32)
            one_t = pool.tile([P, 1], _F32)

            idx0 = pool.tile([128, 448], _I16, tag="idx0")
            idx1 = pool.tile([128, 448], _I16, tag="idx1")
            idx_tiles = {0: idx0, 1: idx1}
            g0 = pool.tile([P, 3584], _F32, tag="g0")
            g1 = pool.tile([P, 3584], _F32, tag="g1")
            g_tiles = {0: g0, 1: g1}
            eqm0 = pool.tile([P, 3584], _F32, tag="eqm0")
            eqm1 = pool.tile([P, 3584], _F32, tag="eqm1")
            eqm_tiles = {0: eqm0, 1: eqm1}
            b_pred = pool.tile([P, 112], _F32)
            lof = pool.tile([P, 112], _F32)
            b_tgt = pool.tile([P, 136], _F32)
            b_gxy = pool.tile([P, 272], _F32)
            t1raw = pool.tile([16, 2176], _I32)
            t2raw = pool.tile([P, 272], _I32)
            msk = pool.tile([P, 136], _F32)

            # ----- phase 0.1: input DMAs, ordered by consumer time ----------
            # sync queue: ind(h0), c_base, iota; scalar: ind(h1), ind-lo, ...
            with tc.tile_wait_until(0.1):
                nc.sync.dma_start(
                    _ap(t1raw[:], [[34, 32], [1, 34]], off=0),
                    _dap(indv[:], [[34, 16], [544, 32], [1, 34]], off=0))
                nc.scalar.dma_start(
                    _ap(t1raw[:], [[34, 32], [1, 34]], off=1088),
                    _dap(indv[:], [[34, 16], [544, 32], [1, 34]],
                         off=512 * 34))
                nc.sync.dma_start(base_t[:], c_base[:])
                nc.sync.dma_start(iota_t[:], c_iota[:])
                for h in range(2):
                    nc.scalar.dma_start(
                        _ap(t2raw[:], [[34, 4], [1, 34]], off=h * 136),
                        _dap(indv[:], [[34, 128], [4352, 4], [1, 34]],
                             off=(512 * h) * 34))

            # ----- phase 0.2: idx math + casts (h0 first), replication ------
            with tc.tile_wait_until(0.2):
                for h in range(2):
                    ev = _ap(t1raw[:], [[34, 32], [2, 17]], off=h * 1088)
                    nc.vector.tensor_scalar(out=ev, in0=ev, scalar1=6,
                                            scalar2=None,
                                            op0=AL.logical_shift_right)
                    nc.vector.tensor_tensor(
                        out=ev, in0=ev,
                        in1=_ap(base_t[:], [[1, 32], [0, 17]]), op=AL.add)
                    it = idx_tiles[h]
                    for (jt, j0, cnt) in _CHUNKS_ALL:
                        nc.vector.tensor_copy(
                            out=_ap(it[0:16, :], [[32, cnt], [1, 32]],
                                    off=jt * 32),
                            in_=_ap(t1raw[:], [[2, cnt], [34, 32]],
                                    off=h * 1088 + 2 * j0))
                    nc.sync.dma_start(it[16:32, 0:448], it[0:16, 0:448])
                # replicate [0:32] to the remaining partition groups
                for h in range(2):
                    it = idx_tiles[h]
                    nc.sync.dma_start(it[32:64, 0:448], it[0:32, 0:448])
                    nc.scalar.dma_start(it[64:96, 0:448], it[0:32, 0:448])
                    nc.sync.dma_start(it[96:128, 0:448], it[0:32, 0:448])

            # ----- phase 0.3: small tensors, lo, active mask ----------------
            with tc.tile_wait_until(0.3):
                for h in range(2):
                    nc.scalar.dma_start(
                        _ap(b_tgt[:], [[17, 4], [1, 17]], off=h * 68),
                        _dap(tgtv[:], [[17, 128], [2176, 4], [1, 17]],
                             off=(512 * h) * 17))
                    nc.scalar.dma_start(
                        _ap(b_gxy[:], [[34, 4], [1, 34]], off=h * 136),
                        _dap(gxyv[:], [[34, 128], [4352, 4], [1, 34]],
                             off=(512 * h) * 34))
                nc.scalar.dma_start(w_t[:], c_w[:])
                nc.scalar.dma_start(one_t[:], c_one[:])
                for h in range(2):
                    nc.scalar.dma_start(
                        _ap(msk[:], [[17, 4], [1, 17]], off=h * 68),
                        _dap(mskv[:], [[17, 128], [2176, 4], [1, 17]],
                             off=(512 * h) * 17))
                nc.vector.tensor_scalar(out=t2raw[:], in0=t2raw[:], scalar1=63,
                                        scalar2=None, op0=AL.bitwise_and)
                for h in range(2):
                    for (jt, j0, cnt) in _CHUNKS_ALL:
                        nc.vector.tensor_copy(
                            out=_ap(lof[:], [[4, cnt], [1, 4]],
                                    off=h * 56 + jt * 4),
                            in_=_ap(t2raw[:], [[2, cnt], [34, 4]],
                                    off=h * 136 + 2 * j0))

            # ----- phase 0.5: eq-mask precompute (hidden under gathers) -----
            with tc.tile_wait_until(0.5):
                for h in range(2):
                    nc.vector.tensor_tensor(
                        out=eqm_tiles[h][:].rearrange("p (a e) -> p a e", e=64),
                        in0=_ap(iota_t[:], [[0, 56], [1, 64]]),
                        in1=_ap(lof[:], [[1, 56], [0, 64]], off=h * 56),
                        op=AL.is_equal)

            # ----- gathers: 4 rounds x 4 queues -----------------------------
            def emit_gather(h, q, s0, ns, ph):
                with tc.tile_wait_until(ph):
                    nc.gpsimd.dma_gather(
                        _ap(g_tiles[h][:], [[64, ns * 4], [1, 64]],
                            off=s0 * 256),
                        outv[h * HS * 64:(h + 1) * HS * 64, :],
                        idx_tiles[h][0:32 * (q + 1), s0 * 32:(s0 + ns) * 32],
                        ns * 512, ns * 512, 64, elem_step=64,
                        queue_num=q,
                    )

            def emit_select(h, s0, ns, ph):
                # eqm <- eqm * g (in place), then reduce over e -> b_pred
                with tc.tile_wait_until(ph):
                    eview = _ap(eqm_tiles[h][:], [[1, ns * 256]], off=s0 * 256)
                    nc.vector.tensor_tensor(
                        out=eview, in0=eview,
                        in1=_ap(g_tiles[h][:], [[1, ns * 256]], off=s0 * 256),
                        op=AL.mult)
                    nc.vector.tensor_reduce(
                        out=_ap(b_pred[:], [[1, ns * 4]], off=h * 56 + s0 * 4),
                        in_=_ap(eqm_tiles[h][:], [[64, ns * 4], [1, 64]],
                                off=s0 * 256),
                        axis=X, op=AL.add)

            for ri, calls in enumerate(_CALL_ROUNDS):
                for ci, (h, q, s0, ns) in enumerate(calls):
                    emit_gather(h, q, s0, ns, 1.0 + ri + 0.01 * ci)

            # ----- phase 0.6: early bone math (target/gt_2d only) -----------
            bg_tv = pool.tile([P, 576], _F32)
            xy2 = pool.tile([P, 96], _F32)
            vw = pool.tile([P, 96], _F32)
            rn = pool.tile([P, 32], _F32)
            gv = pool.tile([P, 96], _F32)
            with tc.tile_wait_until(0.6):
                msum = pool.tile([P, 8], _F32)
                nc.vector.tensor_reduce(out=msum[:],
                                        in_=_ap(msk[:], [[17, 8], [1, 17]]),
                                        axis=X, op=AL.add)
                nc.vector.tensor_scalar(out=msum[:], in0=msum[:], scalar1=0.0,
                                        scalar2=None, op0=AL.is_equal)
                for e, runs in enumerate((_RUNS_E1, _RUNS_E2)):
                    for (b0, ln, q0, st) in runs:
                        j0 = _JL[q0]
                        nc.vector.tensor_copy(
                            out=_ap(bg_tv[:], [[8, ln], [4, 2], [1, 4]],
                                    off=e * 96 + b0 * 8),
                            in_=_ap(b_tgt[:], [[st, ln], [68, 2], [17, 4]],
                                    off=j0))
                        nc.vector.tensor_copy(
                            out=_ap(bg_tv[:], [[16, ln], [8, 2], [4, 2], [1, 4]],
                                    off=192 + e * 192 + b0 * 16),
                            in_=_ap(b_gxy[:], [[2 * st, ln], [1, 2], [136, 2],
                                               [34, 4]],
                                    off=2 * j0))
                n96 = [[1, 96]]
                vis = pool.tile([P, 96], _F32)
                v2 = pool.tile([P, 96], _F32)
                nc.vector.tensor_scalar(out=vis[:], in0=_ap(bg_tv[:], n96, off=0),
                                        scalar1=0.5, scalar2=None, op0=AL.is_gt)
                nc.vector.tensor_scalar(out=v2[:], in0=_ap(bg_tv[:], n96, off=96),
                                        scalar1=0.5, scalar2=None, op0=AL.is_gt)
                nc.vector.tensor_tensor(out=vis[:], in0=vis[:], in1=v2[:],
                                        op=AL.mult)
                # vw = vis * w  (fold bone weight into the visibility gate)
                nc.vector.tensor_tensor(
                    out=vw[:].rearrange("p (a b) -> p a b", a=12),
                    in0=vis[:].rearrange("p (a b) -> p a b", a=12),
                    in1=_ap(w_t[:], [[1, 12], [0, 8]]), op=AL.mult)
                dx = pool.tile([P, 96], _F32)
                dy = pool.tile([P, 96], _F32)
                nc.vector.tensor_tensor(
                    out=dx[:].rearrange("p (a b) -> p a b", a=12),
                    in0=_ap(bg_tv[:], [[16, 12], [1, 8]], off=192),
                    in1=_ap(bg_tv[:], [[16, 12], [1, 8]], off=384),
                    op=AL.subtract)
                nc.vector.tensor_tensor(
                    out=dy[:].rearrange("p (a b) -> p a b", a=12),
                    in0=_ap(bg_tv[:], [[16, 12], [1, 8]], off=200),
                    in1=_ap(bg_tv[:], [[16, 12], [1, 8]], off=392),
                    op=AL.subtract)
                nc.vector.tensor_tensor(out=dx[:], in0=dx[:], in1=dx[:],
                                        op=AL.mult)
                nc.vector.tensor_tensor(out=dy[:], in0=dy[:], in1=dy[:],
                                        op=AL.mult)
                nc.vector.tensor_tensor(out=xy2[:], in0=dx[:], in1=dy[:],
                                        op=AL.add)
                # num = bones visible per group; rn = 1/max(num, 1)
                num = pool.tile([P, 32], _F32)
                nc.vector.tensor_reduce(
                    out=_ap(num[:], [[8, 2], [1, 8]]),
                    in_=_ap(vis[:], [[32, 2], [1, 8], [8, 4]]),
                    axis=X, op=AL.add)
                nc.vector.tensor_reduce(
                    out=_ap(num[:], [[8, 2], [1, 8]], off=16),
                    in_=_ap(vis[:], [[16, 2], [1, 8], [8, 2]], off=64),
                    axis=X, op=AL.add)
                nc.vector.tensor_scalar(out=num[:], in0=num[:], scalar1=1.0,
                                        scalar2=None, op0=AL.max)
                nc.vector.reciprocal(out=rn[:], in_=num[:])
                # rn_m = rn * active-sample mask; gv = vis * rn_m (per bone)
                rn_m = pool.tile([P, 32], _F32)
                nc.vector.tensor_tensor(out=rn_m[:], in0=rn[:],
                                        in1=_ap(msum[:], [[0, 4], [1, 8]]),
                                        op=AL.mult)
                nc.vector.tensor_tensor(
                    out=_ap(gv[:], [[32, 2], [8, 4], [1, 8]]),
                    in0=_ap(vis[:], [[32, 2], [8, 4], [1, 8]]),
                    in1=_ap(rn_m[:], [[8, 2], [0, 4], [1, 8]]), op=AL.mult)
                nc.vector.tensor_tensor(
                    out=_ap(gv[:], [[16, 2], [8, 2], [1, 8]], off=64),
                    in0=_ap(vis[:], [[16, 2], [8, 2], [1, 8]], off=64),
                    in1=_ap(rn_m[:], [[8, 2], [0, 2], [1, 8]], off=16),
                    op=AL.mult)

            # ----- selects, interleaved per round ---------------------------
            for ri, calls in enumerate(_CALL_ROUNDS):
                for ci, (h, q, s0, ns) in enumerate(calls):
                    emit_select(h, s0, ns, 1.5 + ri + 0.01 * ci)

            # ----- late bone math (needs pred) ------------------------------
            with tc.tile_wait_until(6.0):
                n96 = [[1, 96]]
                pd = pool.tile([P, 96], _F32)
                for (b0, ln, p1, s1, p2, s2) in _RUNS_D:
                    nc.vector.tensor_tensor(
                        out=_ap(pd[:], [[8, ln], [4, 2], [1, 4]], off=b0 * 8),
                        in0=_ap(b_pred[:], [[4 * s1, ln], [56, 2], [1, 4]],
                                off=p1 * 4),
                        in1=_ap(b_pred[:], [[4 * s2, ln], [56, 2], [1, 4]],
                                off=p2 * 4),
                        op=AL.subtract)
                d2 = pool.tile([P, 96], _F32)
                nc.vector.tensor_tensor(out=d2[:], in0=pd[:], in1=pd[:],
                                        op=AL.mult)
                nc.vector.tensor_tensor(out=d2[:], in0=d2[:], in1=xy2[:],
                                        op=AL.add)
                # gate = (d2 > 0) * gv, off the sqrt critical path
                gt = pool.tile([P, 96], _F32)
                nc.vector.tensor_scalar(out=gt[:], in0=d2[:], scalar1=0.0,
                                        scalar2=None, op0=AL.is_gt)
                nc.vector.tensor_tensor(out=gt[:], in0=gt[:], in1=gv[:],
                                        op=AL.mult)
                ell = pool.tile([P, 96], _F32)
                nc.scalar.sqrt(out=ell[:], in_=d2[:])
                nc.vector.tensor_tensor(out=ell[:], in0=ell[:], in1=vw[:],
                                        op=AL.mult)
                # per-group mean E = sum_l * rn
                sum_l = pool.tile([P, 32], _F32)
                nc.vector.tensor_reduce(
                    out=_ap(sum_l[:], [[8, 2], [1, 8]]),
                    in_=_ap(ell[:], [[32, 2], [1, 8], [8, 4]]),
                    axis=X, op=AL.add)
                nc.vector.tensor_reduce(
                    out=_ap(sum_l[:], [[8, 2], [1, 8]], off=16),
                    in_=_ap(ell[:], [[16, 2], [1, 8], [8, 2]], off=64),
                    axis=X, op=AL.add)
                e_t = pool.tile([P, 32], _F32)
                nc.vector.tensor_tensor(out=e_t[:], in0=sum_l[:], in1=rn[:],
                                        op=AL.mult)
                # eb = (ell - E_bcast)^2 * gate;  per-lane sums; PE total
                eb = pool.tile([P, 96], _F32)
                nc.vector.tensor_tensor(
                    out=_ap(eb[:], [[32, 2], [8, 4], [1, 8]]),
                    in0=_ap(ell[:], [[32, 2], [8, 4], [1, 8]]),
                    in1=_ap(e_t[:], [[8, 2], [0, 4], [1, 8]]), op=AL.subtract)
                nc.vector.tensor_tensor(
                    out=_ap(eb[:], [[16, 2], [8, 2], [1, 8]], off=64),
                    in0=_ap(ell[:], [[16, 2], [8, 2], [1, 8]], off=64),
                    in1=_ap(e_t[:], [[8, 2], [0, 2], [1, 8]], off=16),
                    op=AL.subtract)
                nc.vector.tensor_tensor(out=eb[:], in0=eb[:], in1=eb[:],
                                        op=AL.mult)
                nc.vector.tensor_tensor(out=eb[:], in0=eb[:], in1=gt[:],
                                        op=AL.mult)
                pl = pool.tile([P, 8], _F32)
                nc.vector.tensor_reduce(out=pl[:],
                                        in_=_ap(eb[:], [[1, 8], [8, 12]]),
                                        axis=X, op=AL.add)
                ps = psum_pool.tile([1, 8], _F32, space="PSUM")
                nc.tensor.matmul(out=ps[:], lhsT=one_t[:], rhs=pl[:],
                                 start=True, stop=True)
                tot = pool.tile([1, 1], _F32)
                nc.vector.tensor_reduce(out=tot[:], in_=ps[:], axis=X, op=AL.add)
                nc.sync.dma_start(res[:], tot[0:1, :])
    nc.compile()
    return nc


_NC_CACHE = None
LAST_RESULTS = None


def kernel(output, mask, ind, target, gt_2d):
    global _NC_CACHE, LAST_RESULTS
    if _NC_CACHE is None:
        _NC_CACHE = _build_nc()
    nc = _NC_CACHE

    output = np.ascontiguousarray(np.asarray(output), dtype=np.float32)
    mask = np.ascontiguousarray(np.asarray(mask), dtype=np.float32)
    target = np.ascontiguousarray(np.asarray(target), dtype=np.float32)
    gt_2d = np.ascontiguousarray(np.asarray(gt_2d), dtype=np.float32)
    ind = np.ascontiguousarray(np.asarray(ind))
    if ind.dtype != np.int64:
        ind = ind.astype(np.int64)

    consts = _consts()
    in_maps = []
    for c in range(NCORES):
        sl = slice(c * S, (c + 1) * S)
        in_maps.append({
            "outv": np.ascontiguousarray(output[sl]).reshape(S * 64, 64),
            "indv": np.ascontiguousarray(ind[sl]).view(np.int32).reshape(S, 34),
            "tgtv": np.ascontiguousarray(target[sl]),
            "gxyv": np.ascontiguousarray(gt_2d[sl]).reshape(S, 34),
            "mskv": np.ascontiguousarray(mask[sl]),
            **consts,
        })
    res = run_bass_kernel_spmd(nc, in_maps, core_ids=list(range(NCORES)))
    LAST_RESULTS = res
    total = sum(float(res.results[c]["res"][0, 0]) for c in range(NCORES))
    return np.asarray([_VAR_WEIGHT * total * 0.5 / B], dtype=np.float32)
